# revision 15
# baseline (speedup 1.0000x reference)
"""Trainium2 Bass kernel for nn_Attention_26946624815974.

Computation (B=64, S=2048, D=512):
    h = tanh(tgt @ W_q.T + src @ W_ref.T)        [B, S, D]
    u = einsum('bsd,d->bs', h, v)                [B, 1, S]
    logit = 10 * tanh(u)
    probs = softmax(logit, -1)
    d_prime = probs @ src                        [B, 1, D]
returns (d_prime, probs, logit)

Sharding: data-parallel over batch, 8 batches per core on 8 NeuronCores.
W_q / W_ref / v replicated. No collectives.

Per-core pipeline (per batch):
    - DMA src natively, round to float32r in place (DVE)
    - PE-transpose src blocks -> srcT (d on partitions) for the main GEMM
    - GEMM r.T[d',s] = W_ref.T @ srcT in float32r (full PE rate)
    - ACT tanh with per-partition bias q -> h (float32r)
    - u = v . h as M=1 matmuls accumulated in PSUM
    - t = tanh(u); logit = 10t; e = exp(10t) via PE-transposed t
      (exp fused with per-partition row-sum)
    - sum_e via tiny fp32 matmul; probs = e/sum_e
    - d_prime = (e.T-stationary @ src-native) / sum_e
"""

import sys

if "/opt/trn_rl_repo" not in sys.path:
    sys.path.insert(0, "/opt/trn_rl_repo")

import numpy as np

import concourse.bass as bass
import concourse.mybir as mybir
import concourse.tile as tile
from concourse import bacc, bass_utils
from concourse.masks import make_identity

F32 = mybir.dt.float32
F32R = mybir.dt.float32r
F16 = mybir.dt.float16
TANH = mybir.ActivationFunctionType.Tanh
EXP = mybir.ActivationFunctionType.Exp

B, S, D = 64, 2048, 512
N_CORES = 8
BL = B // N_CORES  # local batches per core
NSB = S // 128  # 16 s-blocks per batch
NC4 = 4  # chunks of 128 in D; also s-chunks of 512

_built = {}


def build(reps=1, gemm_mult=1, trans_mult=1):
    key = (reps, gemm_mult, trans_mult)
    if key in _built:
        return _built[key]
    nc = bacc.Bacc("TRN2", target_bir_lowering=False, debug=False)
    src = nc.dram_tensor("src", [BL, S, D], F32, kind="ExternalInput").ap()
    tgt = nc.dram_tensor("tgt", [BL, D], F32, kind="ExternalInput").ap()
    wq = nc.dram_tensor("wq", [D, D], F32, kind="ExternalInput").ap()
    wref = nc.dram_tensor("wref", [D, D], F32, kind="ExternalInput").ap()
    v = nc.dram_tensor("v", [1, D], F32, kind="ExternalInput").ap()
    dp_o = nc.dram_tensor("dp", [BL, D], F32, kind="ExternalOutput").ap()
    probs_o = nc.dram_tensor("probs", [BL, S], F32, kind="ExternalOutput").ap()
    logit_o = nc.dram_tensor("logit", [BL, S], F32, kind="ExternalOutput").ap()

    with tile.TileContext(nc) as tc:
        _body(tc, src, tgt, wq, wref, v, dp_o, probs_o, logit_o, reps, gemm_mult, trans_mult)
    nc.compile()
    _built[key] = nc
    return nc


def _body(tc, src, tgt, wq, wref, v, dp_o, probs_o, logit_o, reps, gemm_mult=1, trans_mult=1):
    nc = tc.nc
    from contextlib import ExitStack

    with ExitStack() as ctx:
        const = ctx.enter_context(tc.tile_pool(name="const", bufs=1))
        ident = const.tile([128, 128], F32)
        make_identity(nc, ident[:])
        ident_h = const.tile([128, 128], F16)
        nc.vector.tensor_copy(ident_h[:], ident[:])
        ones_col = const.tile([128, 1], F32)
        nc.gpsimd.memset(ones_col[:], 1.0)
        ones_row = const.tile([1, 128], F32)
        nc.gpsimd.memset(ones_row[:], 1.0)
        wrefT = const.tile([128, NC4, D], F16)  # wrefT[:,k,d'] = W_ref[d', k*128+p]
        q_sb = const.tile([128, NC4, BL], F32)  # q.T chunks
        v_r = const.tile([128, NC4], F32R)

        # ---------- setup: transpose W_ref, W_q, tgt, v; compute q ----------
        with (
            tc.tile_pool(name="setup_sb", bufs=1) as ssb,
            tc.tile_pool(name="setup_ps", bufs=2, space="PSUM") as sps,
        ):
            wn = ssb.tile([128, NC4, D], F32)  # W_ref native row-blocks
            qn = ssb.tile([128, NC4, D], F32)  # W_q native row-blocks
            for j in range(NC4):
                nc.sync.dma_start(wn[:, j], wref[j * 128 : (j + 1) * 128, :])
                nc.sync.dma_start(qn[:, j], wq[j * 128 : (j + 1) * 128, :])
            wqT = ssb.tile([128, NC4, D], F32R)
            for k in range(NC4):
                psw = sps.tile([128, D], F32, tag="sps")
                for j in range(NC4):
                    nc.tensor.transpose(
                        psw[:, j * 128 : (j + 1) * 128],
                        wn[:, j, k * 128 : (k + 1) * 128],
                        ident[:],
                    )
                nc.scalar.copy(wrefT[:, k], psw[:])
                psq = sps.tile([128, D], F32, tag="sps")
                for j in range(NC4):
                    nc.tensor.transpose(
                        psq[:, j * 128 : (j + 1) * 128],
                        qn[:, j, k * 128 : (k + 1) * 128],
                        ident[:],
                    )
                nc.scalar.copy(wqT[:, k], psq[:])

            tg = ssb.tile([BL, D], F32)
            nc.sync.dma_start(tg[:], tgt[:])
            tgtT = ssb.tile([128, NC4, BL], F32R)
            pst = sps.tile([128, D], F32, tag="sps")
            for k in range(NC4):
                nc.tensor.transpose(
                    pst[:, k * BL : (k + 1) * BL],
                    tg[0:BL, k * 128 : (k + 1) * 128],
                    ident[0:BL, 0:BL],
                )
            for k in range(NC4):
                nc.vector.tensor_copy(tgtT[:, k], pst[:, k * BL : (k + 1) * BL])

            vt = ssb.tile([1, D], F32)
            nc.sync.dma_start(vt[:], v[:])
            psv = sps.tile([128, NC4], F32, tag="sps2")
            for m in range(NC4):
                nc.tensor.transpose(
                    psv[:, m : m + 1],
                    vt[0:1, m * 128 : (m + 1) * 128],
                    ident[0:1, 0:1],
                )
            nc.vector.tensor_copy(v_r[:], psv[:])

            # q.T[d', b] = sum_d W_q[d', d] tgt[b, d]
            for m in range(NC4):
                psqq = sps.tile([128, BL], F32, tag="sps2")
                for k in range(NC4):
                    nc.tensor.matmul(
                        psqq[:],
                        wqT[:, k, m * 128 : (m + 1) * 128],
                        tgtT[:, k],
                        start=(k == 0),
                        stop=(k == NC4 - 1),
                    )
                nc.scalar.copy(q_sb[:, m], psqq[:])

        # ---------- main pipeline ----------
        nat_p = ctx.enter_context(tc.tile_pool(name="nat", bufs=2))
        bounce_p = ctx.enter_context(tc.tile_pool(name="bounce", bufs=3))
        srcT_p = ctx.enter_context(tc.tile_pool(name="srcT", bufs=2))
        h_p = ctx.enter_context(tc.tile_pool(name="h", bufs=4))
        t_p = ctx.enter_context(tc.tile_pool(name="t", bufs=2))
        eT_p = ctx.enter_context(tc.tile_pool(name="eT", bufs=2))
        sm_p = ctx.enter_context(tc.tile_pool(name="sm", bufs=4))
        out_p = ctx.enter_context(tc.tile_pool(name="outp", bufs=2))
        psT_p = ctx.enter_context(tc.tile_pool(name="psT", bufs=2, space="PSUM"))
        psR_p = ctx.enter_context(tc.tile_pool(name="psR", bufs=2, space="PSUM"))
        psU_p = ctx.enter_context(tc.tile_pool(name="psU", bufs=1, space="PSUM"))
        psB_p = ctx.enter_context(tc.tile_pool(name="psB", bufs=2, space="PSUM"))

        for rep in range(reps):
            for b in range(BL):
                # HWDGE load into f32 bounce, DVE-round into f32r natr
                natr = nat_p.tile([128, NSB, D], F16, tag="nat")
                for c in range(NC4):
                    bounce = bounce_p.tile([128, 4, D], F32, tag="bounce")
                    nc.sync.dma_start(
                        bounce[:],
                        src[b, c * 512 : (c + 1) * 512, :].rearrange(
                            "(si p) d -> p si d", p=128
                        ),
                    )
                    nc.vector.tensor_copy(natr[:, c * 4 : (c + 1) * 4, :], bounce[:])

                srcT = srcT_p.tile([128, NC4, S], F16, tag="srcT")
                t_sb = t_p.tile([1, S], F32, tag="t")
                for c in range(NC4):
                    # transpose 16 blocks [128,128] -> srcT[:, k, c*512:+512]
                    for k in range(NC4):
                        psT = psT_p.tile([128, 512], F16, tag="psT")
                        for _tm in range(trans_mult):
                            for si in range(4):
                                nc.tensor.transpose(
                                    psT[:, si * 128 : (si + 1) * 128],
                                    natr[:, c * 4 + si, k * 128 : (k + 1) * 128],
                                    ident_h[:],
                                )
                        dst = srcT[:, k, c * 512 : (c + 1) * 512]
                        if k % 2 == 0:
                            nc.scalar.copy(dst, psT[:])
                        else:
                            nc.vector.tensor_copy(dst, psT[:])
                    # GEMM (all fp16 matmuls back-to-back), tanh, then u (f32r)
                    psU = psU_p.tile([1, 512], F32, tag="psU")
                    hs = []
                    for m in range(NC4):
                        psR = psR_p.tile([128, 512], F32, tag="psR")
                        for _gm in range(gemm_mult):
                            for k in range(NC4):
                                nc.tensor.matmul(
                                    psR[:],
                                    wrefT[:, k, m * 128 : (m + 1) * 128],
                                    srcT[:, k, c * 512 : (c + 1) * 512],
                                    start=(k == 0),
                                    stop=(k == NC4 - 1),
                                )
                        h = h_p.tile([128, 512], F32R, tag=f"h{m}")
                        nc.scalar.activation(
                            h[:], psR[:], TANH, bias=q_sb[:, m, b : b + 1]
                        )
                        hs.append(h)
                    for m in range(NC4):
                        nc.tensor.matmul(
                            psU[:],
                            v_r[:, m : m + 1],
                            hs[m][:],
                            start=(m == 0),
                            stop=(m == NC4 - 1),
                        )
                    nc.scalar.activation(t_sb[0:1, c * 512 : (c + 1) * 512], psU[:], TANH)

                # ---- phase B ----
                logit_sb = out_p.tile([1, S], F32, tag="logit")
                nc.vector.tensor_scalar_mul(logit_sb[:], t_sb[:], 10.0)
                nc.sync.dma_start(logit_o[b : b + 1, :], logit_sb[:])

                psTT = psB_p.tile([128, NSB], F32, tag="psB")
                for j in range(NSB):
                    nc.tensor.transpose(
                        psTT[:, j : j + 1],
                        t_sb[0:1, j * 128 : (j + 1) * 128],
                        ident[0:1, 0:1],
                    )
                eT = eT_p.tile([128, NSB], F16, tag="eT")
                rowsum = sm_p.tile([128, 1], F32, tag="rowsum")
                nc.scalar.activation(
                    eT[:], psTT[:], EXP, scale=10.0, accum_out=rowsum[:]
                )
                ps_tot = psB_p.tile([1, 1], F32, tag="psB")
                nc.tensor.matmul(
                    ps_tot[:], rowsum[:], ones_col[:], start=True, stop=True
                )
                inv1 = sm_p.tile([1, 1], F32, tag="inv1")
                nc.vector.reciprocal(inv1[:], ps_tot[:])
                ps_invb = psB_p.tile([128, 1], F32, tag="psB")
                nc.tensor.matmul(
                    ps_invb[:], ones_row[:], inv1[:], start=True, stop=True
                )
                inv_col = sm_p.tile([128, 1], F32, tag="invcol")
                nc.vector.tensor_copy(inv_col[:], ps_invb[:])

                probsT = out_p.tile([128, NSB], F32, tag="probsT")
                nc.vector.tensor_scalar_mul(probsT[:], eT[:], inv_col[:])
                nc.sync.dma_start(
                    probs_o[b].rearrange("(j p) -> p j", p=128), probsT[:]
                )

                ps_dp = psB_p.tile([1, 512], F32, tag="psB")
                for j in range(NSB):
                    nc.tensor.matmul(
                        ps_dp[:],
                        eT[:, j : j + 1],
                        natr[:, j, :],
                        start=(j == 0),
                        stop=(j == NSB - 1),
                    )
                dp_sb = out_p.tile([1, 512], F32, tag="dp")
                nc.vector.tensor_scalar_mul(dp_sb[:], ps_dp[:], inv1[:])
                nc.sync.dma_start(dp_o[b : b + 1, :], dp_sb[:])


def kernel(src, tgt, W_q, W_ref, v):
    nc = build()
    in_maps = []
    for c in range(N_CORES):
        sl = slice(c * BL, (c + 1) * BL)
        in_maps.append(
            {
                "src": np.ascontiguousarray(src[sl], dtype=np.float32),
                "tgt": np.ascontiguousarray(tgt[sl, 0, :], dtype=np.float32),
                "wq": np.ascontiguousarray(W_q, dtype=np.float32),
                "wref": np.ascontiguousarray(W_ref, dtype=np.float32),
                "v": np.ascontiguousarray(v.reshape(1, D), dtype=np.float32),
            }
        )
    res = bass_utils.run_bass_kernel_spmd(nc, in_maps, core_ids=list(range(N_CORES)))
    d_prime = np.concatenate([res.results[c]["dp"] for c in range(N_CORES)], axis=0)
    probs = np.concatenate([res.results[c]["probs"] for c in range(N_CORES)], axis=0)
    logit = np.concatenate([res.results[c]["logit"] for c in range(N_CORES)], axis=0)
    return (
        d_prime.reshape(B, 1, D),
        probs.reshape(B, 1, S),
        logit.reshape(B, 1, S),
    )


# revision 16
# speedup vs baseline: 1.6185x; 1.6185x over previous
"""Trainium2 Bass kernel for nn_Attention_26946624815974.

Computation (B=64, S=2048, D=512):
    h = tanh(tgt @ W_q.T + src @ W_ref.T)        [B, S, D]
    u = einsum('bsd,d->bs', h, v)                [B, 1, S]
    logit = 10 * tanh(u)
    probs = softmax(logit, -1)
    d_prime = probs @ src                        [B, 1, D]
returns (d_prime, probs, logit)

Sharding: data-parallel over batch, 8 batches per core on 8 NeuronCores.
W_q / W_ref / v replicated. No collectives.

Per-core pipeline (per batch):
    - DMA src natively, round to float32r in place (DVE)
    - PE-transpose src blocks -> srcT (d on partitions) for the main GEMM
    - GEMM r.T[d',s] = W_ref.T @ srcT in float32r (full PE rate)
    - ACT tanh with per-partition bias q -> h (float32r)
    - u = v . h as M=1 matmuls accumulated in PSUM
    - t = tanh(u); logit = 10t; e = exp(10t) via PE-transposed t
      (exp fused with per-partition row-sum)
    - sum_e via tiny fp32 matmul; probs = e/sum_e
    - d_prime = (e.T-stationary @ src-native) / sum_e
"""

import sys

if "/opt/trn_rl_repo" not in sys.path:
    sys.path.insert(0, "/opt/trn_rl_repo")

import numpy as np

import concourse.bass as bass
import concourse.mybir as mybir
import concourse.tile as tile
from concourse import bacc, bass_utils
from concourse.masks import make_identity

F32 = mybir.dt.float32
F32R = mybir.dt.float32r
F16 = mybir.dt.float16
TANH = mybir.ActivationFunctionType.Tanh
EXP = mybir.ActivationFunctionType.Exp

B, S, D = 64, 2048, 512
N_CORES = 8
BL = B // N_CORES  # local batches per core
NSB = S // 128  # 16 s-blocks per batch
NC4 = 4  # chunks of 128 in D; also s-chunks of 512

_built = {}


def build(reps=1, gemm_mult=1, trans_mult=1):
    key = (reps, gemm_mult, trans_mult)
    if key in _built:
        return _built[key]
    nc = bacc.Bacc("TRN2", target_bir_lowering=False, debug=False)
    src = nc.dram_tensor("src", [BL, S, D], F32, kind="ExternalInput").ap()
    tgt = nc.dram_tensor("tgt", [BL, D], F32, kind="ExternalInput").ap()
    wq = nc.dram_tensor("wq", [D, D], F32, kind="ExternalInput").ap()
    wref = nc.dram_tensor("wref", [D, D], F32, kind="ExternalInput").ap()
    v = nc.dram_tensor("v", [1, D], F32, kind="ExternalInput").ap()
    dp_o = nc.dram_tensor("dp", [BL, D], F32, kind="ExternalOutput").ap()
    probs_o = nc.dram_tensor("probs", [BL, S], F32, kind="ExternalOutput").ap()
    logit_o = nc.dram_tensor("logit", [BL, S], F32, kind="ExternalOutput").ap()

    with tile.TileContext(nc) as tc:
        _body(tc, src, tgt, wq, wref, v, dp_o, probs_o, logit_o, reps, gemm_mult, trans_mult)
    nc.compile()
    _built[key] = nc
    return nc


def _body(tc, src, tgt, wq, wref, v, dp_o, probs_o, logit_o, reps, gemm_mult=1, trans_mult=1):
    nc = tc.nc
    from contextlib import ExitStack

    with ExitStack() as ctx:
        const = ctx.enter_context(tc.tile_pool(name="const", bufs=1))
        ident = const.tile([128, 128], F32)
        make_identity(nc, ident[:])
        ident_h = const.tile([128, 128], F16)
        nc.vector.tensor_copy(ident_h[:], ident[:])
        ones_col = const.tile([128, 1], F32)
        nc.gpsimd.memset(ones_col[:], 1.0)
        ones_row = const.tile([1, 128], F32)
        nc.gpsimd.memset(ones_row[:], 1.0)
        wrefT = const.tile([128, NC4, D], F16)  # wrefT[:,k,d'] = W_ref[d', k*128+p]
        q_sb = const.tile([128, NC4, BL], F32)  # q.T chunks
        v_r = const.tile([128, NC4], F32R)

        # ---------- setup: transpose W_ref, W_q, tgt, v; compute q ----------
        with (
            tc.tile_pool(name="setup_sb", bufs=1) as ssb,
            tc.tile_pool(name="setup_ps", bufs=2, space="PSUM") as sps,
        ):
            wn = ssb.tile([128, NC4, D], F32)  # W_ref native row-blocks
            qn = ssb.tile([128, NC4, D], F32)  # W_q native row-blocks
            for j in range(NC4):
                nc.sync.dma_start(wn[:, j], wref[j * 128 : (j + 1) * 128, :])
                nc.sync.dma_start(qn[:, j], wq[j * 128 : (j + 1) * 128, :])
            wqT = ssb.tile([128, NC4, D], F32R)
            for k in range(NC4):
                psw = sps.tile([128, D], F32, tag="sps")
                for j in range(NC4):
                    nc.tensor.transpose(
                        psw[:, j * 128 : (j + 1) * 128],
                        wn[:, j, k * 128 : (k + 1) * 128],
                        ident[:],
                    )
                nc.scalar.copy(wrefT[:, k], psw[:])
                psq = sps.tile([128, D], F32, tag="sps")
                for j in range(NC4):
                    nc.tensor.transpose(
                        psq[:, j * 128 : (j + 1) * 128],
                        qn[:, j, k * 128 : (k + 1) * 128],
                        ident[:],
                    )
                nc.scalar.copy(wqT[:, k], psq[:])

            tg = ssb.tile([BL, D], F32)
            nc.sync.dma_start(tg[:], tgt[:])
            tgtT = ssb.tile([128, NC4, BL], F32R)
            pst = sps.tile([128, D], F32, tag="sps")
            for k in range(NC4):
                nc.tensor.transpose(
                    pst[:, k * BL : (k + 1) * BL],
                    tg[0:BL, k * 128 : (k + 1) * 128],
                    ident[0:BL, 0:BL],
                )
            for k in range(NC4):
                nc.vector.tensor_copy(tgtT[:, k], pst[:, k * BL : (k + 1) * BL])

            vt = ssb.tile([1, D], F32)
            nc.sync.dma_start(vt[:], v[:])
            psv = sps.tile([128, NC4], F32, tag="sps2")
            for m in range(NC4):
                nc.tensor.transpose(
                    psv[:, m : m + 1],
                    vt[0:1, m * 128 : (m + 1) * 128],
                    ident[0:1, 0:1],
                )
            nc.vector.tensor_copy(v_r[:], psv[:])

            # q.T[d', b] = sum_d W_q[d', d] tgt[b, d]
            for m in range(NC4):
                psqq = sps.tile([128, BL], F32, tag="sps2")
                for k in range(NC4):
                    nc.tensor.matmul(
                        psqq[:],
                        wqT[:, k, m * 128 : (m + 1) * 128],
                        tgtT[:, k],
                        start=(k == 0),
                        stop=(k == NC4 - 1),
                    )
                nc.scalar.copy(q_sb[:, m], psqq[:])

        # ---------- main pipeline ----------
        nat_p = ctx.enter_context(tc.tile_pool(name="nat", bufs=2))
        bounce_p = ctx.enter_context(tc.tile_pool(name="bounce", bufs=3))
        srcT_p = ctx.enter_context(tc.tile_pool(name="srcT", bufs=2))
        h_p = ctx.enter_context(tc.tile_pool(name="h", bufs=4))
        t_p = ctx.enter_context(tc.tile_pool(name="t", bufs=2))
        eT_p = ctx.enter_context(tc.tile_pool(name="eT", bufs=2))
        sm_p = ctx.enter_context(tc.tile_pool(name="sm", bufs=4))
        out_p = ctx.enter_context(tc.tile_pool(name="outp", bufs=2))
        psT_p = ctx.enter_context(tc.tile_pool(name="psT", bufs=2, space="PSUM"))
        psR_p = ctx.enter_context(tc.tile_pool(name="psR", bufs=2, space="PSUM"))
        psX_p = ctx.enter_context(tc.tile_pool(name="psX", bufs=2, space="PSUM"))

        for rep in range(reps):
            for b in range(BL):
                # HWDGE load into f32 bounce, DVE-round into f32r natr
                natr = nat_p.tile([128, NSB, D], F16, tag="nat")
                for c in range(NC4):
                    bounce = bounce_p.tile([128, 4, D], F32, tag="bounce")
                    nc.sync.dma_start(
                        bounce[:],
                        src[b, c * 512 : (c + 1) * 512, :].rearrange(
                            "(si p) d -> p si d", p=128
                        ),
                    )
                    nc.vector.tensor_copy(natr[:, c * 4 : (c + 1) * 4, :], bounce[:])

                srcT = srcT_p.tile([128, NC4, S], F16, tag="srcT")
                t_sb = t_p.tile([1, S], F32, tag="t")
                for cp in range(2):
                    # transposes for both chunks of the pair
                    for ci in range(2):
                        c = 2 * cp + ci
                        for k in range(NC4):
                            psT = psT_p.tile([128, 512], F16, tag="psT")
                            for _tm in range(trans_mult):
                                for si in range(4):
                                    nc.tensor.transpose(
                                        psT[:, si * 128 : (si + 1) * 128],
                                        natr[:, c * 4 + si, k * 128 : (k + 1) * 128],
                                        ident_h[:],
                                    )
                            dst = srcT[:, k, c * 512 : (c + 1) * 512]
                            if k % 2 == 0:
                                nc.scalar.copy(dst, psT[:])
                            else:
                                nc.vector.tensor_copy(dst, psT[:])
                    # GEMM: each stationary (k,m) streams both chunks of the pair
                    hs = {}
                    for m in range(NC4):
                        psR = psR_p.tile([128, 2, 512], F32, tag="psR")
                        for _gm in range(gemm_mult):
                            for k in range(NC4):
                                for ci in range(2):
                                    c = 2 * cp + ci
                                    nc.tensor.matmul(
                                        psR[:, ci],
                                        wrefT[:, k, m * 128 : (m + 1) * 128],
                                        srcT[:, k, c * 512 : (c + 1) * 512],
                                        start=(k == 0),
                                        stop=(k == NC4 - 1),
                                    )
                        for ci in range(2):
                            h = h_p.tile([128, 512], F32R, tag=f"h{m}{ci}")
                            nc.scalar.activation(
                                h[:], psR[:, ci], TANH, bias=q_sb[:, m, b : b + 1]
                            )
                            hs[(m, ci)] = h
                    for ci in range(2):
                        c = 2 * cp + ci
                        psU = psX_p.tile([1, 512], F32, tag="psX")
                        for m in range(NC4):
                            nc.tensor.matmul(
                                psU[:],
                                v_r[:, m : m + 1],
                                hs[(m, ci)][:],
                                start=(m == 0),
                                stop=(m == NC4 - 1),
                            )
                        nc.scalar.activation(
                            t_sb[0:1, c * 512 : (c + 1) * 512], psU[:], TANH
                        )

                # ---- phase B ----
                logit_sb = out_p.tile([1, S], F32, tag="logit")
                nc.vector.tensor_scalar_mul(logit_sb[:], t_sb[:], 10.0)
                nc.sync.dma_start(logit_o[b : b + 1, :], logit_sb[:])

                psTT = psX_p.tile([128, NSB], F32, tag="psX")
                for j in range(NSB):
                    nc.tensor.transpose(
                        psTT[:, j : j + 1],
                        t_sb[0:1, j * 128 : (j + 1) * 128],
                        ident[0:1, 0:1],
                    )
                eT = eT_p.tile([128, NSB], F16, tag="eT")
                rowsum = sm_p.tile([128, 1], F32, tag="rowsum")
                nc.scalar.activation(
                    eT[:], psTT[:], EXP, scale=10.0, accum_out=rowsum[:]
                )
                ps_tot = psX_p.tile([1, 1], F32, tag="psX")
                nc.tensor.matmul(
                    ps_tot[:], rowsum[:], ones_col[:], start=True, stop=True
                )
                inv1 = sm_p.tile([1, 1], F32, tag="inv1")
                nc.vector.reciprocal(inv1[:], ps_tot[:])
                ps_invb = psX_p.tile([128, 1], F32, tag="psX")
                nc.tensor.matmul(
                    ps_invb[:], ones_row[:], inv1[:], start=True, stop=True
                )
                inv_col = sm_p.tile([128, 1], F32, tag="invcol")
                nc.vector.tensor_copy(inv_col[:], ps_invb[:])

                probsT = out_p.tile([128, NSB], F32, tag="probsT")
                nc.vector.tensor_scalar_mul(probsT[:], eT[:], inv_col[:])
                nc.sync.dma_start(
                    probs_o[b].rearrange("(j p) -> p j", p=128), probsT[:]
                )

                ps_dp = psX_p.tile([1, 512], F32, tag="psX")
                for j in range(NSB):
                    nc.tensor.matmul(
                        ps_dp[:],
                        eT[:, j : j + 1],
                        natr[:, j, :],
                        start=(j == 0),
                        stop=(j == NSB - 1),
                    )
                dp_sb = out_p.tile([1, 512], F32, tag="dp")
                nc.vector.tensor_scalar_mul(dp_sb[:], ps_dp[:], inv1[:])
                nc.sync.dma_start(dp_o[b : b + 1, :], dp_sb[:])


def kernel(src, tgt, W_q, W_ref, v):
    nc = build()
    in_maps = []
    for c in range(N_CORES):
        sl = slice(c * BL, (c + 1) * BL)
        in_maps.append(
            {
                "src": np.ascontiguousarray(src[sl], dtype=np.float32),
                "tgt": np.ascontiguousarray(tgt[sl, 0, :], dtype=np.float32),
                "wq": np.ascontiguousarray(W_q, dtype=np.float32),
                "wref": np.ascontiguousarray(W_ref, dtype=np.float32),
                "v": np.ascontiguousarray(v.reshape(1, D), dtype=np.float32),
            }
        )
    res = bass_utils.run_bass_kernel_spmd(nc, in_maps, core_ids=list(range(N_CORES)))
    d_prime = np.concatenate([res.results[c]["dp"] for c in range(N_CORES)], axis=0)
    probs = np.concatenate([res.results[c]["probs"] for c in range(N_CORES)], axis=0)
    logit = np.concatenate([res.results[c]["logit"] for c in range(N_CORES)], axis=0)
    return (
        d_prime.reshape(B, 1, D),
        probs.reshape(B, 1, S),
        logit.reshape(B, 1, S),
    )
